# revision 2
# baseline (speedup 1.0000x reference)
"""Multi-head causal attention block (B=4, T=2048, C=1024, H=16) on 8 TRN2 cores.

Sharding: core c handles batch b = c // 2 and head-group hg = c % 2 (8 heads).
Each core computes q/k/v for its 8 heads from x[b], runs causal attention, and
produces a partial output-projection y_partial[b] = attnout @ out_w[rows_hg].
Host sums the two head-group partials per batch (fp32) and adds out_b.

All matmul operands are bf16 (fp32 matmul is 2-pass on the PE; bf16 is
single-pass => 2x tensor throughput), accumulation stays fp32 in PSUM.
Feature-major layout: x is fed as xT = x[b].T so the QKV projection needs no
on-device transposes; q/k come out d-major with head pairs stacked on
partitions 0-63 / 64-127 (row-packed K=64 score matmuls via tile_position);
v comes out token-major which is the lhsT layout for attn@V. A ones column
appended to each V tile yields the softmax denominator on row 64 of the
attn@V output.

Pipeline: attention runs as a stream of 1-k-block groups with a one-group
lookahead (scores of group g+1 issue before attn@V of group g) so the PE
never head-blocks waiting on the ScalarE exp. Q/K projection of pair p+1 and
the output projection are interleaved into the stream as PE filler. Softmax
normalization uses a K=1 outer-product matmul to broadcast 1/den across
partitions (no DRAM bounce).
"""

import os
import sys
from collections import deque
from contextlib import ExitStack

import numpy as np
import ml_dtypes

for _p in ("/opt/trn_rl_repo", "/root/.axon_site/_ro/trn_rl_repo"):
    if os.path.isdir(_p) and _p not in sys.path:
        sys.path.insert(0, _p)

import concourse.bass as bass
import concourse.bacc as bacc
import concourse.mybir as mybir
import concourse.tile as tile
from concourse.bass_utils import run_bass_kernel_spmd

B, T, C, H = 4, 2048, 1024, 16
D = C // H  # 64
N_CORES = 8
HG = 2  # head groups per batch (cores per batch)
HPG = H // HG  # 8 heads per core
PAIRS = HPG // 2  # 4 head pairs per core
TB = T // 128  # 16 token blocks
QT = T // 512  # 4 q tiles
CT = C // 128  # 8 contraction tiles
FP32 = mybir.dt.float32
BF16 = mybir.dt.bfloat16
F32R = mybir.dt.float32r
BF16_NP = ml_dtypes.bfloat16
SCALE = 1.0 / np.sqrt(np.float32(D))

_program_cache = {}


def build_program():
    nc = bacc.Bacc("TRN2", target_bir_lowering=False, debug=False, num_devices=N_CORES)

    xT = nc.declare_dram_parameter("xT", [C, T], BF16, isOutput=False)
    wq = nc.declare_dram_parameter("wq", [C, 512], BF16, isOutput=False)
    wk = nc.declare_dram_parameter("wk", [C, 512], BF16, isOutput=False)
    wv = nc.declare_dram_parameter("wv", [C, 512], BF16, isOutput=False)
    bq = nc.declare_dram_parameter("bq", [128, PAIRS], FP32, isOutput=False)
    bk = nc.declare_dram_parameter("bk", [128, PAIRS], FP32, isOutput=False)
    bv = nc.declare_dram_parameter("bv", [1, 512], FP32, isOutput=False)
    wo = nc.declare_dram_parameter("wo", [512, C], BF16, isOutput=False)
    maskp = nc.declare_dram_parameter("mask", [128, 128], BF16, isOutput=False)
    y = nc.declare_dram_parameter("y", [T, C], BF16, isOutput=True)

    Exp = mybir.ActivationFunctionType.Exp

    with tile.TileContext(nc) as tc, ExitStack() as ctx:
        persist = ctx.enter_context(tc.tile_pool(name="persist", bufs=1))

        mask_sb = persist.tile([128, 128], BF16, name="mask_sb", tag="mask_sb")
        nc.sync.dma_start(mask_sb, maskp[:, :])
        bq_sb = persist.tile([128, PAIRS], FP32, name="bq_sb", tag="bq_sb")
        nc.sync.dma_start(bq_sb, bq[:, :])
        bk_sb = persist.tile([128, PAIRS], FP32, name="bk_sb", tag="bk_sb")
        nc.sync.dma_start(bk_sb, bk[:, :])
        ones_sb = persist.tile([1, 64], FP32, name="ones_sb", tag="ones_sb")
        nc.vector.memset(ones_sb, 1.0)

        # V with interleaved ones columns: per token-block, [128, 8*65] where
        # group h holds V[:, h*64:(h+1)*64] | 1.
        v_sb = [
            persist.tile([128, HPG * 65], BF16, name=f"v_sb{i}", tag=f"v_sb{i}")
            for i in range(TB)
        ]
        # q/k per pair, d-major, heads stacked on partitions (0-63 / 64-127)
        qst = [
            persist.tile([128, T], BF16, name=f"qst{p}", tag=f"qst{p}")
            for p in range(PAIRS)
        ]
        kst = [
            persist.tile([128, T], BF16, name=f"kst{p}", tag=f"kst{p}")
            for p in range(PAIRS)
        ]
        onorm = [
            persist.tile([128, T], BF16, name=f"onorm{p}", tag=f"on{p}")
            for p in range(PAIRS)
        ]
        wo_sb = [
            persist.tile([128, C], BF16, name=f"wo_sb{p}", tag=f"wo{p}")
            for p in range(PAIRS)
        ]
        xt_sb = [
            persist.tile([128, T], BF16, name=f"xt_sb{i}", tag=f"xt{i}")
            for i in range(CT)
        ]
        wv_sb = [
            persist.tile([128, 512], BF16, name=f"wv_sb{i}", tag=f"wv{i}")
            for i in range(CT)
        ]
        bv_bc = persist.tile([128, 512], FP32, name="bv_bc", tag="bv_bc")
        nc.sync.dma_start(bv_bc, bv[:, :].to_broadcast([128, 512]))

        for i in range(CT):
            nc.sync.dma_start(wv_sb[i], wv[i * 128 : (i + 1) * 128, :])
        # xT chunked (wide DMAs fan out across many HW queues and blow the
        # per-instruction sync-wait limit on consumers), column-major so the
        # first token blocks of every contraction tile land first
        for c in range(T // 512):
            for i in range(CT):
                nc.sync.dma_start(
                    xt_sb[i][:, c * 512 : (c + 1) * 512],
                    xT[i * 128 : (i + 1) * 128, c * 512 : (c + 1) * 512],
                )
        for p in range(PAIRS):
            for c in range(C // 512):
                nc.sync.dma_start(
                    wo_sb[p][:, c * 512 : (c + 1) * 512],
                    wo[p * 128 : (p + 1) * 128, c * 512 : (c + 1) * 512],
                )

        wqk_pool = ctx.enter_context(tc.tile_pool(name="wqk", bufs=2))
        p_pool = ctx.enter_context(tc.tile_pool(name="pexp", bufs=6))
        small_pool = ctx.enter_context(tc.tile_pool(name="small", bufs=4))

        # ---------------- V pass (scoped PSUM pool) ----------------
        vctx = ExitStack()
        vpsum = vctx.enter_context(tc.tile_pool(name="vpsum", bufs=3, space="PSUM"))
        for tb in range(TB):
            pv = vpsum.tile([128, 512], FP32, name="pv", tag="pv")
            for ci in range(CT):
                nc.tensor.matmul(
                    pv,
                    xt_sb[ci][:, tb * 128 : (tb + 1) * 128],
                    wv_sb[ci],
                    start=(ci == 0),
                    stop=(ci == CT - 1),
                )
            vt = v_sb[tb].rearrange("p (h e) -> p h e", e=65)
            nc.vector.tensor_add(
                vt[:, :, 0:64],
                pv.rearrange("p (h e) -> p h e", e=64),
                bv_bc.rearrange("p (h e) -> p h e", e=64),
            )
            nc.vector.memset(vt[:, :, 64:65], 1.0)
        vctx.close()

        # ---------------- attention-phase PSUM pools ----------------
        # budget: sps 2x[128,1024] = 4 banks, outps 2x[128,512] = 2, pq 1,
        # bc 1 -> 8 banks exactly.
        pqp = ctx.enter_context(tc.tile_pool(name="pqp", bufs=1, space="PSUM"))
        spsum = ctx.enter_context(tc.tile_pool(name="spsum", bufs=2, space="PSUM"))
        apsum = ctx.enter_context(tc.tile_pool(name="apsum", bufs=1, space="PSUM"))
        bcp = ctx.enter_context(tc.tile_pool(name="bcp", bufs=1, space="PSUM"))

        # ---- Q/K projection emitted as closures (interleavable) ----
        def emit_qk(pr):
            closures = []
            for wdram, bias_sb, dst in ((wq, bq_sb, qst[pr]), (wk, bk_sb, kst[pr])):
                wt = []

                def load_w(wdram=wdram, wt=wt, pr=pr):
                    for ci in range(CT):
                        w_t = wqk_pool.tile(
                            [128, 128], BF16, name=f"w_t{ci}", tag=f"w{ci}"
                        )
                        nc.sync.dma_start(
                            w_t,
                            wdram[ci * 128 : (ci + 1) * 128, pr * 128 : (pr + 1) * 128],
                        )
                        wt.append(w_t)

                closures.append(load_w)

                def chunk(qt, wt=wt, bias_sb=bias_sb, dst=dst, pr=pr):
                    def go():
                        pq = pqp.tile([128, 512], FP32, name="pq", tag="pq")
                        for ci in range(CT):
                            nc.tensor.matmul(
                                pq,
                                wt[ci],
                                xt_sb[ci][:, qt * 512 : (qt + 1) * 512],
                                start=(ci == 0),
                                stop=(ci == CT - 1),
                            )
                        nc.vector.tensor_scalar_add(
                            dst[:, qt * 512 : (qt + 1) * 512],
                            pq,
                            bias_sb[:, pr : pr + 1],
                        )

                    return go

                closures.extend(chunk(qt) for qt in range(QT))
            return closures

        def outproj_unit(tb, nh):
            def go():
                yp = pqp.tile([128, 512], FP32, name="yp", tag="pq")
                for p2 in range(PAIRS):
                    nc.tensor.matmul(
                        yp,
                        onorm[p2][:, tb * 128 : (tb + 1) * 128],
                        wo_sb[p2][:, nh * 512 : (nh + 1) * 512],
                        start=(p2 == 0),
                        stop=(p2 == PAIRS - 1),
                    )
                ys = small_pool.tile([128, 512], BF16, name="ys", tag="ys")
                if (tb + nh) % 2:
                    nc.scalar.copy(ys, yp)
                else:
                    nc.vector.tensor_copy(ys, yp)
                nc.sync.dma_start(
                    y[tb * 128 : (tb + 1) * 128, nh * 512 : (nh + 1) * 512], ys
                )

            return go

        # Q/K for pair 0 runs up front; later pairs interleave into attention.
        for cl in emit_qk(0):
            cl()

        misc = deque()
        pending = None

        for pr in range(PAIRS):
            if pr + 1 < PAIRS:
                misc.extend(emit_qk(pr + 1))
            for qt in range(QT):
                nkb = 4 * qt + 4
                outps = [
                    apsum.tile([128, 512], FP32, name=f"outp{hh}", tag=f"av{hh}")
                    for hh in (0, 1)
                ]
                # 1-k-block groups: off-diagonal (full 512 q cols), then the 4
                # diagonal sub-blocks (column-trimmed): (kb, ncols, qcol0, diag)
                subs = [(kb, 512, 0, False) for kb in range(4 * qt)]
                subs += [(4 * qt + j, 512 - 128 * j, 128 * j, True) for j in range(4)]

                for kb, ncols, qcol0, diag in subs:
                    # scores for both head-halves packed into one psum tile:
                    # hh0 at cols [0, ncols), hh1 at cols [512, 512+ncols)
                    sps = spsum.tile([128, 1024], FP32, name="sps", tag="sc")
                    for hh in (0, 1):
                        nc.tensor.matmul(
                            sps[:, hh * 512 : hh * 512 + ncols],
                            kst[pr][hh * 64 : hh * 64 + 64, kb * 128 : (kb + 1) * 128],
                            qst[pr][
                                hh * 64 : hh * 64 + 64,
                                qt * 512 + qcol0 : qt * 512 + qcol0 + ncols,
                            ],
                            start=True,
                            stop=True,
                            tile_position=(hh * 64, 0),
                        )
                    pexp = p_pool.tile([128, 1024], BF16, name="pexp", tag="p")
                    if ncols == 512:
                        nc.scalar.activation(pexp, sps, Exp, scale=float(SCALE))
                    else:
                        for hh in (0, 1):
                            nc.scalar.activation(
                                pexp[:, hh * 512 : hh * 512 + ncols],
                                sps[:, hh * 512 : hh * 512 + ncols],
                                Exp,
                                scale=float(SCALE),
                            )
                    if diag:
                        # zero the strictly-upper triangle of the 128-wide
                        # diagonal window (post-exp 0/1 mask)
                        for hh in (0, 1):
                            nc.vector.tensor_mul(
                                pexp[:, hh * 512 : hh * 512 + 128],
                                pexp[:, hh * 512 : hh * 512 + 128],
                                mask_sb,
                            )
                    if pending is not None:
                        pending()

                    def attnv(
                        pexp=pexp,
                        kb=kb,
                        ncols=ncols,
                        qcol0=qcol0,
                        outps=outps,
                        pr=pr,
                        first=(kb == 0),
                        last=(kb == nkb - 1),
                    ):
                        vs = v_sb[kb].rearrange("p (h e) -> p h e", e=65)
                        for hh in (0, 1):
                            nc.tensor.matmul(
                                outps[hh][0:65, qcol0 : qcol0 + ncols],
                                vs[:, 2 * pr + hh, :],
                                pexp[:, hh * 512 : hh * 512 + ncols],
                                start=first,
                                stop=last,
                            )

                    pending = attnv
                    if misc:
                        misc.popleft()()

                pending()
                pending = None

                # normalize: den sits on psum row 64 (ones column). Immediate
                # part (no PE): reciprocal + raw-copy free the outps banks;
                # deferred part (PE broadcast matmul + DVE mul) goes to misc.
                for hh in (0, 1):
                    rec = small_pool.tile([1, 512], FP32, name="rec", tag="rec")
                    nc.vector.reciprocal(rec, outps[hh][64:65, :])
                    raw = small_pool.tile([64, 512], BF16, name="raw", tag="raw")
                    nc.scalar.copy(raw, outps[hh][0:64, :])

                    def norm_tail(rec=rec, raw=raw, pr=pr, qt=qt, hh=hh):
                        bc = bcp.tile([64, 512], FP32, name="bc", tag="bc")
                        nc.tensor.matmul(
                            bc,
                            ones_sb.bitcast(F32R),
                            rec.bitcast(F32R),
                            start=True,
                            stop=True,
                        )
                        nc.vector.tensor_mul(
                            onorm[pr][
                                hh * 64 : hh * 64 + 64, qt * 512 : (qt + 1) * 512
                            ],
                            raw,
                            bc,
                        )

                    misc.append(norm_tail)

                if pr == PAIRS - 1:
                    for tb in range(qt * 4, qt * 4 + 4):
                        for nh in (0, 1):
                            misc.append(outproj_unit(tb, nh))

        while misc:
            misc.popleft()()

    if not nc.is_finalized():
        nc.finalize()
    return nc


def shard_inputs(x, qkv_w, qkv_b, out_w):
    """Build the 8 per-core input maps (host-side bf16 casts)."""
    x = np.asarray(x, dtype=np.float32)
    qkv_w = np.asarray(qkv_w, dtype=np.float32)
    qkv_b = np.asarray(qkv_b, dtype=np.float32)
    out_w = np.asarray(out_w, dtype=np.float32)

    # 0/1 lower-triangular keep-mask for the post-exp diagonal-window zeroing
    mask = (
        (np.arange(128)[:, None] <= np.arange(128)[None, :])
        .astype(BF16_NP)
    )

    in_maps = []
    for core in range(N_CORES):
        b, hg = core // HG, core % HG
        col0 = hg * 512
        wq_np = np.ascontiguousarray(qkv_w[:, col0 : col0 + 512]).astype(BF16_NP)
        wk_np = np.ascontiguousarray(qkv_w[:, C + col0 : C + col0 + 512]).astype(
            BF16_NP
        )
        wv_np = np.ascontiguousarray(
            qkv_w[:, 2 * C + col0 : 2 * C + col0 + 512]
        ).astype(BF16_NP)
        bq_np = np.ascontiguousarray(qkv_b[col0 : col0 + 512].reshape(PAIRS, 128).T)
        bk_np = np.ascontiguousarray(
            qkv_b[C + col0 : C + col0 + 512].reshape(PAIRS, 128).T
        )
        bv_np = np.ascontiguousarray(
            qkv_b[2 * C + col0 : 2 * C + col0 + 512].reshape(1, 512)
        )
        wo_np = np.ascontiguousarray(out_w[col0 : col0 + 512, :]).astype(BF16_NP)
        xT_np = np.ascontiguousarray(x[b].T).astype(BF16_NP)
        in_maps.append(
            {
                "xT": xT_np,
                "wq": wq_np,
                "wk": wk_np,
                "wv": wv_np,
                "bq": bq_np,
                "bk": bk_np,
                "bv": bv_np,
                "wo": wo_np,
                "mask": mask,
            }
        )
    return in_maps


def kernel(x, qkv_w, qkv_b, out_w, out_b, _trace=False, _tmpdir=None):
    if "nc" not in _program_cache:
        _program_cache["nc"] = build_program()
    nc = _program_cache["nc"]

    in_maps = shard_inputs(x, qkv_w, qkv_b, out_w)
    res = run_bass_kernel_spmd(
        nc,
        in_maps,
        core_ids=list(range(N_CORES)),
        trace=_trace,
        tmpdir=_tmpdir,
    )
    _program_cache["last_results"] = res

    out_b = np.asarray(out_b, dtype=np.float32)
    y = np.empty((B, T, C), dtype=np.float32)
    for b in range(B):
        y[b] = (
            res.results[2 * b]["y"].astype(np.float32)
            + res.results[2 * b + 1]["y"].astype(np.float32)
            + out_b
        )
    return y


# revision 7
# speedup vs baseline: 2.8926x; 2.8926x over previous
"""Multi-head causal attention block (B=4, T=2048, C=1024, H=16) on 8 TRN2 cores.

Sharding: core c handles batch b = c // 2 and head-group hg = c % 2 (8 heads).
Each core computes q/k/v for its 8 heads from x[b], runs causal attention, and
produces a partial output-projection y_partial[b] = attnout @ out_w[rows_hg].
Host sums the two head-group partials per batch (fp32) and adds out_b.

All matmul operands are bf16 (fp32 matmul is 2-pass on the PE; bf16 is
single-pass => 2x tensor throughput), accumulation stays fp32 in PSUM.
Feature-major layout: x is fed as xT = x[b].T so the QKV projection needs no
on-device transposes; q/k come out d-major with head pairs stacked on
partitions 0-63 / 64-127 (row-packed K=64 score matmuls via tile_position);
v comes out token-major which is the lhsT layout for attn@V. A ones column
appended to each V tile yields the softmax denominator on row 64 of the
attn@V output.

Pipeline: attention runs as a stream of 1-k-block groups with a one-group
lookahead (scores of group g+1 issue before attn@V of group g) so the PE
never head-blocks waiting on the ScalarE exp. Q/K projection of pair p+1 and
the output projection are interleaved into the stream as PE filler. Softmax
normalization uses a K=1 outer-product matmul to broadcast 1/den across
partitions (no DRAM bounce).
"""

import os
import sys
from collections import deque
from contextlib import ExitStack

import numpy as np
import ml_dtypes

for _p in ("/opt/trn_rl_repo", "/root/.axon_site/_ro/trn_rl_repo"):
    if os.path.isdir(_p) and _p not in sys.path:
        sys.path.insert(0, _p)

import concourse.bass as bass
import concourse.bacc as bacc
import concourse.mybir as mybir
import concourse.tile as tile
from concourse.bass_utils import run_bass_kernel_spmd

B, T, C, H = 4, 2048, 1024, 16
D = C // H  # 64
N_CORES = 8
HG = 2  # head groups per batch (cores per batch)
HPG = H // HG  # 8 heads per core
PAIRS = HPG // 2  # 4 head pairs per core
TB = T // 128  # 16 token blocks
QT = T // 512  # 4 q tiles
CT = C // 128  # 8 contraction tiles
FP32 = mybir.dt.float32
BF16 = mybir.dt.bfloat16
F32R = mybir.dt.float32r
BF16_NP = ml_dtypes.bfloat16
SCALE = 1.0 / np.sqrt(np.float32(D))

_program_cache = {}


def build_program():
    nc = bacc.Bacc("TRN2", target_bir_lowering=False, debug=False, num_devices=N_CORES)

    xT = nc.declare_dram_parameter("xT", [C, T], BF16, isOutput=False)
    wq = nc.declare_dram_parameter("wq", [C, 512], BF16, isOutput=False)
    wk = nc.declare_dram_parameter("wk", [C, 512], BF16, isOutput=False)
    wv = nc.declare_dram_parameter("wv", [C, 512], BF16, isOutput=False)
    bq = nc.declare_dram_parameter("bq", [128, PAIRS], FP32, isOutput=False)
    bk = nc.declare_dram_parameter("bk", [128, PAIRS], FP32, isOutput=False)
    bv = nc.declare_dram_parameter("bv", [1, 512], FP32, isOutput=False)
    wo = nc.declare_dram_parameter("wo", [512, C], BF16, isOutput=False)
    maskp = nc.declare_dram_parameter("mask", [128, 128], BF16, isOutput=False)
    y = nc.declare_dram_parameter("y", [T, C], BF16, isOutput=True)

    Exp = mybir.ActivationFunctionType.Exp

    with tile.TileContext(nc) as tc, ExitStack() as ctx:
        persist = ctx.enter_context(tc.tile_pool(name="persist", bufs=1))

        mask_sb = persist.tile([128, 128], BF16, name="mask_sb", tag="mask_sb")
        nc.sync.dma_start(mask_sb, maskp[:, :])
        bq_sb = persist.tile([128, PAIRS], FP32, name="bq_sb", tag="bq_sb")
        nc.sync.dma_start(bq_sb, bq[:, :])
        bk_sb = persist.tile([128, PAIRS], FP32, name="bk_sb", tag="bk_sb")
        nc.sync.dma_start(bk_sb, bk[:, :])
        ones_sb = persist.tile([1, 64], BF16, name="ones_sb", tag="ones_sb")
        nc.vector.memset(ones_sb, 1.0)

        # V with interleaved ones columns: per token-block, [128, 8*65] where
        # group h holds V[:, h*64:(h+1)*64] | 1.
        v_sb = [
            persist.tile([128, HPG * 65], BF16, name=f"v_sb{i}", tag=f"v_sb{i}")
            for i in range(TB)
        ]
        # q/k per pair, d-major, heads stacked on partitions (0-63 / 64-127)
        qst = [
            persist.tile([128, T], BF16, name=f"qst{p}", tag=f"qst{p}")
            for p in range(PAIRS)
        ]
        kst = [
            persist.tile([128, T], BF16, name=f"kst{p}", tag=f"kst{p}")
            for p in range(PAIRS)
        ]
        onorm = [
            persist.tile([128, T], BF16, name=f"onorm{p}", tag=f"on{p}")
            for p in range(PAIRS)
        ]
        wo_sb = [
            persist.tile([128, C], BF16, name=f"wo_sb{p}", tag=f"wo{p}")
            for p in range(PAIRS)
        ]
        xt_sb = [
            persist.tile([128, T], BF16, name=f"xt_sb{i}", tag=f"xt{i}")
            for i in range(CT)
        ]
        wv_sb = [
            persist.tile([128, 512], BF16, name=f"wv_sb{i}", tag=f"wv{i}")
            for i in range(CT)
        ]
        bv_bc = persist.tile([128, 512], FP32, name="bv_bc", tag="bv_bc")
        nc.sync.dma_start(bv_bc, bv[:, :].to_broadcast([128, 512]))

        for i in range(CT):
            nc.sync.dma_start(wv_sb[i], wv[i * 128 : (i + 1) * 128, :])
        # xT chunked (wide DMAs fan out across many HW queues and blow the
        # per-instruction sync-wait limit on consumers), column-major so the
        # first token blocks of every contraction tile land first
        for c in range(T // 512):
            for i in range(CT):
                nc.sync.dma_start(
                    xt_sb[i][:, c * 512 : (c + 1) * 512],
                    xT[i * 128 : (i + 1) * 128, c * 512 : (c + 1) * 512],
                )
        for p in range(PAIRS):
            for c in range(C // 512):
                nc.sync.dma_start(
                    wo_sb[p][:, c * 512 : (c + 1) * 512],
                    wo[p * 128 : (p + 1) * 128, c * 512 : (c + 1) * 512],
                )

        wqk_pool = ctx.enter_context(tc.tile_pool(name="wqk", bufs=2))
        p_pool = ctx.enter_context(tc.tile_pool(name="pexp", bufs=6))
        small_pool = ctx.enter_context(tc.tile_pool(name="small", bufs=4))

        # ---------------- V pass (scoped PSUM pool) ----------------
        vctx = ExitStack()
        vpsum = vctx.enter_context(tc.tile_pool(name="vpsum", bufs=3, space="PSUM"))
        for tb in range(TB):
            pv = vpsum.tile([128, 512], FP32, name="pv", tag="pv")
            for ci in range(CT):
                nc.tensor.matmul(
                    pv,
                    xt_sb[ci][:, tb * 128 : (tb + 1) * 128],
                    wv_sb[ci],
                    start=(ci == 0),
                    stop=(ci == CT - 1),
                )
            vt = v_sb[tb].rearrange("p (h e) -> p h e", e=65)
            nc.vector.tensor_add(
                vt[:, :, 0:64],
                pv.rearrange("p (h e) -> p h e", e=64),
                bv_bc.rearrange("p (h e) -> p h e", e=64),
            )
            nc.vector.memset(vt[:, :, 64:65], 1.0)
        vctx.close()

        # ---------------- attention-phase PSUM pools ----------------
        # budget: sps 2x[128,1024] = 4 banks, outps 2x[128,512] = 2, pq 1,
        # bc 1 -> 8 banks exactly.
        pqp = ctx.enter_context(tc.tile_pool(name="pqp", bufs=1, space="PSUM"))
        spsum = ctx.enter_context(tc.tile_pool(name="spsum", bufs=2, space="PSUM"))
        apsum = ctx.enter_context(tc.tile_pool(name="apsum", bufs=1, space="PSUM"))
        bcp = ctx.enter_context(tc.tile_pool(name="bcp", bufs=1, space="PSUM"))

        # ---- Q/K projection emitted as closures (interleavable) ----
        def emit_qk(pr):
            closures = []
            for wdram, bias_sb, dst in ((wq, bq_sb, qst[pr]), (wk, bk_sb, kst[pr])):
                wt = []

                def load_w(wdram=wdram, wt=wt, pr=pr):
                    for ci in range(CT):
                        w_t = wqk_pool.tile(
                            [128, 128], BF16, name=f"w_t{ci}", tag=f"w{ci}"
                        )
                        nc.sync.dma_start(
                            w_t,
                            wdram[ci * 128 : (ci + 1) * 128, pr * 128 : (pr + 1) * 128],
                        )
                        wt.append(w_t)

                closures.append(load_w)

                def chunk(qt, wt=wt, bias_sb=bias_sb, dst=dst, pr=pr):
                    def go():
                        pq = pqp.tile([128, 512], FP32, name="pq", tag="pq")
                        for ci in range(CT):
                            nc.tensor.matmul(
                                pq,
                                wt[ci],
                                xt_sb[ci][:, qt * 512 : (qt + 1) * 512],
                                start=(ci == 0),
                                stop=(ci == CT - 1),
                            )
                        nc.vector.tensor_scalar_add(
                            dst[:, qt * 512 : (qt + 1) * 512],
                            pq,
                            bias_sb[:, pr : pr + 1],
                        )

                    return go

                closures.extend(chunk(qt) for qt in range(QT))
            return closures

        def outproj_unit(tb, nh):
            def go():
                yp = pqp.tile([128, 512], FP32, name="yp", tag="pq")
                for p2 in range(PAIRS):
                    nc.tensor.matmul(
                        yp,
                        onorm[p2][:, tb * 128 : (tb + 1) * 128],
                        wo_sb[p2][:, nh * 512 : (nh + 1) * 512],
                        start=(p2 == 0),
                        stop=(p2 == PAIRS - 1),
                    )
                ys = small_pool.tile([128, 512], BF16, name="ys", tag="ys")
                if (tb + nh) % 2:
                    nc.scalar.copy(ys, yp)
                else:
                    nc.vector.tensor_copy(ys, yp)
                nc.sync.dma_start(
                    y[tb * 128 : (tb + 1) * 128, nh * 512 : (nh + 1) * 512], ys
                )

            return go

        # Q/K for pair 0 runs up front; later pairs interleave into attention.
        for cl in emit_qk(0):
            cl()

        misc = deque()
        pending = None

        for pr in range(PAIRS):
            if pr + 1 < PAIRS:
                misc.extend(emit_qk(pr + 1))
            for qt in range(QT):
                nkb = 4 * qt + 4
                outps = [
                    apsum.tile([128, 512], FP32, name=f"outp{hh}", tag=f"av{hh}")
                    for hh in (0, 1)
                ]
                # 1-k-block groups: off-diagonal (full 512 q cols), then the 4
                # diagonal sub-blocks (column-trimmed): (kb, ncols, qcol0, diag)
                subs = [(kb, 512, 0, False) for kb in range(4 * qt)]
                subs += [(4 * qt + j, 512 - 128 * j, 128 * j, True) for j in range(4)]

                for kb, ncols, qcol0, diag in subs:
                    # scores for both head-halves packed into one psum tile:
                    # hh0 at cols [0, ncols), hh1 at cols [512, 512+ncols)
                    sps = spsum.tile([128, 1024], FP32, name="sps", tag="sc")
                    for hh in (0, 1):
                        nc.tensor.matmul(
                            sps[:, hh * 512 : hh * 512 + ncols],
                            kst[pr][hh * 64 : hh * 64 + 64, kb * 128 : (kb + 1) * 128],
                            qst[pr][
                                hh * 64 : hh * 64 + 64,
                                qt * 512 + qcol0 : qt * 512 + qcol0 + ncols,
                            ],
                            start=True,
                            stop=True,
                            tile_position=(hh * 64, 0),
                        )
                    pexp = p_pool.tile([128, 1024], BF16, name="pexp", tag="p")
                    if ncols == 512:
                        nc.scalar.activation(pexp, sps, Exp, scale=float(SCALE))
                    else:
                        for hh in (0, 1):
                            nc.scalar.activation(
                                pexp[:, hh * 512 : hh * 512 + ncols],
                                sps[:, hh * 512 : hh * 512 + ncols],
                                Exp,
                                scale=float(SCALE),
                            )
                    if diag:
                        # zero the strictly-upper triangle of the 128-wide
                        # diagonal window (post-exp 0/1 mask)
                        for hh in (0, 1):
                            nc.vector.tensor_mul(
                                pexp[:, hh * 512 : hh * 512 + 128],
                                pexp[:, hh * 512 : hh * 512 + 128],
                                mask_sb,
                            )
                    if pending is not None:
                        pending()

                    def attnv(
                        pexp=pexp,
                        kb=kb,
                        ncols=ncols,
                        qcol0=qcol0,
                        outps=outps,
                        pr=pr,
                        first=(kb == 0),
                        last=(kb == nkb - 1),
                    ):
                        vs = v_sb[kb].rearrange("p (h e) -> p h e", e=65)
                        for hh in (0, 1):
                            nc.tensor.matmul(
                                outps[hh][0:65, qcol0 : qcol0 + ncols],
                                vs[:, 2 * pr + hh, :],
                                pexp[:, hh * 512 : hh * 512 + ncols],
                                start=first,
                                stop=last,
                            )

                    pending = attnv
                    if misc:
                        misc.popleft()()

                pending()
                pending = None

                # normalize: den sits on psum row 64 (ones column). Immediate
                # part (no PE): den-copy (ACT) + raw-copy (DVE) free the outps
                # banks. Deferred part (broadcast matmul + reciprocal over the
                # broadcast [64,512] — 64 lanes, NOT the serial [1,512] row —
                # + DVE mul) goes to misc.
                for hh in (0, 1):
                    den = small_pool.tile([1, 512], BF16, name="den", tag="den")
                    nc.scalar.copy(den, outps[hh][64:65, :])
                    raw = small_pool.tile([64, 512], BF16, name="raw", tag="raw")
                    nc.vector.tensor_copy(raw, outps[hh][0:64, :])

                    def norm_tail(den=den, raw=raw, pr=pr, qt=qt, hh=hh):
                        bc = bcp.tile([64, 512], FP32, name="bc", tag="bc")
                        nc.tensor.matmul(
                            bc,
                            ones_sb,
                            den,
                            start=True,
                            stop=True,
                        )
                        rbc = small_pool.tile([64, 512], FP32, name="rbc", tag="rbc")
                        nc.vector.reciprocal(rbc, bc)
                        nc.vector.tensor_mul(
                            onorm[pr][
                                hh * 64 : hh * 64 + 64, qt * 512 : (qt + 1) * 512
                            ],
                            raw,
                            rbc,
                        )

                    misc.append(norm_tail)

                if pr == PAIRS - 1:
                    for tb in range(qt * 4, qt * 4 + 4):
                        for nh in (0, 1):
                            misc.append(outproj_unit(tb, nh))

        while misc:
            misc.popleft()()

    if not nc.is_finalized():
        nc.finalize()
    return nc


def shard_inputs(x, qkv_w, qkv_b, out_w):
    """Build the 8 per-core input maps (host-side bf16 casts)."""
    x = np.asarray(x, dtype=np.float32)
    qkv_w = np.asarray(qkv_w, dtype=np.float32)
    qkv_b = np.asarray(qkv_b, dtype=np.float32)
    out_w = np.asarray(out_w, dtype=np.float32)

    # 0/1 lower-triangular keep-mask for the post-exp diagonal-window zeroing
    mask = (
        (np.arange(128)[:, None] <= np.arange(128)[None, :])
        .astype(BF16_NP)
    )

    in_maps = []
    for core in range(N_CORES):
        b, hg = core // HG, core % HG
        col0 = hg * 512
        wq_np = np.ascontiguousarray(qkv_w[:, col0 : col0 + 512]).astype(BF16_NP)
        wk_np = np.ascontiguousarray(qkv_w[:, C + col0 : C + col0 + 512]).astype(
            BF16_NP
        )
        wv_np = np.ascontiguousarray(
            qkv_w[:, 2 * C + col0 : 2 * C + col0 + 512]
        ).astype(BF16_NP)
        bq_np = np.ascontiguousarray(qkv_b[col0 : col0 + 512].reshape(PAIRS, 128).T)
        bk_np = np.ascontiguousarray(
            qkv_b[C + col0 : C + col0 + 512].reshape(PAIRS, 128).T
        )
        bv_np = np.ascontiguousarray(
            qkv_b[2 * C + col0 : 2 * C + col0 + 512].reshape(1, 512)
        )
        wo_np = np.ascontiguousarray(out_w[col0 : col0 + 512, :]).astype(BF16_NP)
        xT_np = np.ascontiguousarray(x[b].T).astype(BF16_NP)
        in_maps.append(
            {
                "xT": xT_np,
                "wq": wq_np,
                "wk": wk_np,
                "wv": wv_np,
                "bq": bq_np,
                "bk": bk_np,
                "bv": bv_np,
                "wo": wo_np,
                "mask": mask,
            }
        )
    return in_maps


def kernel(x, qkv_w, qkv_b, out_w, out_b, _trace=False, _tmpdir=None):
    if "nc" not in _program_cache:
        _program_cache["nc"] = build_program()
    nc = _program_cache["nc"]

    in_maps = shard_inputs(x, qkv_w, qkv_b, out_w)
    res = run_bass_kernel_spmd(
        nc,
        in_maps,
        core_ids=list(range(N_CORES)),
        trace=_trace,
        tmpdir=_tmpdir,
    )
    _program_cache["last_results"] = res

    out_b = np.asarray(out_b, dtype=np.float32)
    y = np.empty((B, T, C), dtype=np.float32)
    for b in range(B):
        y[b] = (
            res.results[2 * b]["y"].astype(np.float32)
            + res.results[2 * b + 1]["y"].astype(np.float32)
            + out_b
        )
    return y


# revision 15
# speedup vs baseline: 2.9924x; 1.0345x over previous
"""Multi-head causal attention block (B=4, T=2048, C=1024, H=16) on 8 TRN2 cores.

Sharding: core c handles batch b = c // 2 and head-group hg = c % 2 (8 heads).
Each core computes q/k/v for its 8 heads from x[b], runs causal attention, and
produces a partial output-projection y_partial[b] = attnout @ out_w[rows_hg].
Host sums the two head-group partials per batch (fp32) and adds out_b.

All matmul operands are bf16 (fp32 matmul is 2-pass on the PE; bf16 is
single-pass => 2x tensor throughput), accumulation stays fp32 in PSUM.
Feature-major layout: x is fed as xT = x[b].T so the QKV projection needs no
on-device transposes; q/k come out d-major with head pairs stacked on
partitions 0-63 / 64-127 (row-packed K=64 score matmuls via tile_position);
v comes out token-major which is the lhsT layout for attn@V. A ones column
appended to each V tile yields the softmax denominator on row 64 of the
attn@V output.

Pipeline: attention runs as a stream of 1-k-block groups with a one-group
lookahead (scores of group g+1 issue before attn@V of group g) so the PE
never head-blocks waiting on the ScalarE exp. Q/K projection of pair p+1 and
the output projection are interleaved into the stream as PE filler. Softmax
normalization uses a K=1 outer-product matmul to broadcast 1/den across
partitions (no DRAM bounce).
"""

import os
import sys
from collections import deque
from contextlib import ExitStack

import numpy as np
import ml_dtypes

for _p in ("/opt/trn_rl_repo", "/root/.axon_site/_ro/trn_rl_repo"):
    if os.path.isdir(_p) and _p not in sys.path:
        sys.path.insert(0, _p)

import concourse.bass as bass
import concourse.bacc as bacc
import concourse.mybir as mybir
import concourse.tile as tile
from concourse.bass_utils import run_bass_kernel_spmd

B, T, C, H = 4, 2048, 1024, 16
D = C // H  # 64
N_CORES = 8
HG = 2  # head groups per batch (cores per batch)
HPG = H // HG  # 8 heads per core
PAIRS = HPG // 2  # 4 head pairs per core
TB = T // 128  # 16 token blocks
QT = T // 512  # 4 q tiles
CT = C // 128  # 8 contraction tiles
FP32 = mybir.dt.float32
BF16 = mybir.dt.bfloat16
F32R = mybir.dt.float32r
BF16_NP = ml_dtypes.bfloat16
SCALE = 1.0 / np.sqrt(np.float32(D))

_program_cache = {}


def build_program():
    nc = bacc.Bacc("TRN2", target_bir_lowering=False, debug=False, num_devices=N_CORES)

    xT = nc.declare_dram_parameter("xT", [C, T], BF16, isOutput=False)
    wq = nc.declare_dram_parameter("wq", [C, 512], BF16, isOutput=False)
    wk = nc.declare_dram_parameter("wk", [C, 512], BF16, isOutput=False)
    wv = nc.declare_dram_parameter("wv", [C, 512], BF16, isOutput=False)
    bq = nc.declare_dram_parameter("bq", [128, PAIRS], FP32, isOutput=False)
    bk = nc.declare_dram_parameter("bk", [128, PAIRS], FP32, isOutput=False)
    bv = nc.declare_dram_parameter("bv", [1, 512], FP32, isOutput=False)
    wo = nc.declare_dram_parameter("wo", [512, C], BF16, isOutput=False)
    maskp = nc.declare_dram_parameter("mask", [128, 128], BF16, isOutput=False)
    y = nc.declare_dram_parameter("y", [T, C], BF16, isOutput=True)

    Exp = mybir.ActivationFunctionType.Exp

    with tile.TileContext(nc) as tc, ExitStack() as ctx:
        persist = ctx.enter_context(tc.tile_pool(name="persist", bufs=1))

        mask_sb = persist.tile([128, 128], BF16, name="mask_sb", tag="mask_sb")
        nc.sync.dma_start(mask_sb, maskp[:, :])
        bq_sb = persist.tile([128, PAIRS], FP32, name="bq_sb", tag="bq_sb")
        nc.sync.dma_start(bq_sb, bq[:, :])
        bk_sb = persist.tile([128, PAIRS], FP32, name="bk_sb", tag="bk_sb")
        nc.sync.dma_start(bk_sb, bk[:, :])
        ones_sb = persist.tile([1, 64], BF16, name="ones_sb", tag="ones_sb")
        nc.vector.memset(ones_sb, 1.0)

        # V with interleaved ones columns: per token-block, [128, 8*65] where
        # group h holds V[:, h*64:(h+1)*64] | 1.
        v_sb = [
            persist.tile([128, HPG * 65], BF16, name=f"v_sb{i}", tag=f"v_sb{i}")
            for i in range(TB)
        ]
        # q/k per pair, d-major, heads stacked on partitions (0-63 / 64-127)
        qst = [
            persist.tile([128, T], BF16, name=f"qst{p}", tag=f"qst{p}")
            for p in range(PAIRS)
        ]
        kst = [
            persist.tile([128, T], BF16, name=f"kst{p}", tag=f"kst{p}")
            for p in range(PAIRS)
        ]
        onorm = [
            persist.tile([128, T], BF16, name=f"onorm{p}", tag=f"on{p}")
            for p in range(PAIRS)
        ]
        wo_sb = [
            persist.tile([128, C], BF16, name=f"wo_sb{p}", tag=f"wo{p}")
            for p in range(PAIRS)
        ]
        xt_sb = [
            persist.tile([128, T], BF16, name=f"xt_sb{i}", tag=f"xt{i}")
            for i in range(CT)
        ]
        wv_sb = [
            persist.tile([128, 512], BF16, name=f"wv_sb{i}", tag=f"wv{i}")
            for i in range(CT)
        ]
        bv_bc = persist.tile([128, 512], FP32, name="bv_bc", tag="bv_bc")
        nc.sync.dma_start(bv_bc, bv[:, :].to_broadcast([128, 512]))

        # DMA issue order matters: the V pass needs wv + the first xT column
        # chunk, so those go first. xT is chunked (wide DMAs fan out across
        # many HW queues and blow the per-instruction sync-wait limit on
        # consumers), column-major so early token blocks land first.
        for i in range(CT):
            nc.sync.dma_start(wv_sb[i], wv[i * 128 : (i + 1) * 128, :])
        for c in range(T // 512):
            for i in range(CT):
                nc.sync.dma_start(
                    xt_sb[i][:, c * 512 : (c + 1) * 512],
                    xT[i * 128 : (i + 1) * 128, c * 512 : (c + 1) * 512],
                )
        for p in range(PAIRS):
            for c in range(C // 512):
                nc.sync.dma_start(
                    wo_sb[p][:, c * 512 : (c + 1) * 512],
                    wo[p * 128 : (p + 1) * 128, c * 512 : (c + 1) * 512],
                )

        wqk_pool = ctx.enter_context(tc.tile_pool(name="wqk", bufs=2))
        p_pool = ctx.enter_context(tc.tile_pool(name="pexp", bufs=6))
        small_pool = ctx.enter_context(tc.tile_pool(name="small", bufs=4))

        # ---------------- V pass (scoped PSUM pool) ----------------
        vctx = ExitStack()
        vpsum = vctx.enter_context(tc.tile_pool(name="vpsum", bufs=3, space="PSUM"))
        for tb in range(TB):
            pv = vpsum.tile([128, 512], FP32, name="pv", tag="pv")
            for ci in range(CT):
                nc.tensor.matmul(
                    pv,
                    xt_sb[ci][:, tb * 128 : (tb + 1) * 128],
                    wv_sb[ci],
                    start=(ci == 0),
                    stop=(ci == CT - 1),
                )
            vt = v_sb[tb].rearrange("p (h e) -> p h e", e=65)
            nc.vector.tensor_add(
                vt[:, :, 0:64],
                pv.rearrange("p (h e) -> p h e", e=64),
                bv_bc.rearrange("p (h e) -> p h e", e=64),
            )
            nc.vector.memset(vt[:, :, 64:65], 1.0)
        vctx.close()

        # ---------------- attention-phase PSUM pools ----------------
        # budget: sps 2x[128,1024] = 4 banks, outps 2x[128,512] = 2, pq 1,
        # bc 1 -> 8 banks exactly.
        pqp = ctx.enter_context(tc.tile_pool(name="pqp", bufs=1, space="PSUM"))
        spsum = ctx.enter_context(tc.tile_pool(name="spsum", bufs=2, space="PSUM"))
        apsum = ctx.enter_context(tc.tile_pool(name="apsum", bufs=1, space="PSUM"))
        bcp = ctx.enter_context(tc.tile_pool(name="bcp", bufs=1, space="PSUM"))

        # ---- Q/K projection emitted as closures (interleavable) ----
        def emit_qk(pr):
            closures = []
            for wdram, bias_sb, dst in ((wq, bq_sb, qst[pr]), (wk, bk_sb, kst[pr])):
                wt = []

                def load_w(wdram=wdram, wt=wt, pr=pr):
                    for ci in range(CT):
                        w_t = wqk_pool.tile(
                            [128, 128], BF16, name=f"w_t{ci}", tag=f"w{ci}"
                        )
                        nc.sync.dma_start(
                            w_t,
                            wdram[ci * 128 : (ci + 1) * 128, pr * 128 : (pr + 1) * 128],
                        )
                        wt.append(w_t)

                closures.append(load_w)

                def chunk(qt, wt=wt, bias_sb=bias_sb, dst=dst, pr=pr):
                    def go():
                        pq = pqp.tile([128, 512], FP32, name="pq", tag="pq")
                        for ci in range(CT):
                            nc.tensor.matmul(
                                pq,
                                wt[ci],
                                xt_sb[ci][:, qt * 512 : (qt + 1) * 512],
                                start=(ci == 0),
                                stop=(ci == CT - 1),
                            )
                        nc.vector.tensor_scalar_add(
                            dst[:, qt * 512 : (qt + 1) * 512],
                            pq,
                            bias_sb[:, pr : pr + 1],
                        )

                    return go

                closures.extend(chunk(qt) for qt in range(QT))
            return closures

        def outproj_unit(tb, nh):
            def go():
                yp = pqp.tile([128, 512], FP32, name="yp", tag="pq")
                for p2 in range(PAIRS):
                    nc.tensor.matmul(
                        yp,
                        onorm[p2][:, tb * 128 : (tb + 1) * 128],
                        wo_sb[p2][:, nh * 512 : (nh + 1) * 512],
                        start=(p2 == 0),
                        stop=(p2 == PAIRS - 1),
                    )
                ys = small_pool.tile([128, 512], BF16, name="ys", tag="ys")
                nc.vector.tensor_copy(ys, yp)
                nc.sync.dma_start(
                    y[tb * 128 : (tb + 1) * 128, nh * 512 : (nh + 1) * 512], ys
                )

            return go

        # Q/K for pair 0 runs up front; later pairs interleave into attention.
        for cl in emit_qk(0):
            cl()

        misc = deque()
        pending = None
        qt_finalize = None

        def make_finalize(pr, qt, outps, last_attnv):
            """Flush the tile's last attn@V, then free the outps banks with
            den/raw copies (DVE only), deferring the broadcast-matmul +
            approx-reciprocal + mul to the misc stream. Called after the NEXT
            tile's first score group so the PE pipeline never drains."""

            def go():
                last_attnv()
                for hh in (0, 1):
                    den = small_pool.tile([1, 512], BF16, name="den", tag="den")
                    nc.vector.tensor_copy(den, outps[hh][64:65, :])
                    raw = small_pool.tile([64, 512], BF16, name="raw", tag="raw")
                    nc.vector.tensor_copy(raw, outps[hh][0:64, :])

                    def norm_tail(den=den, raw=raw, pr=pr, qt=qt, hh=hh):
                        bc = bcp.tile([64, 512], FP32, name="bc", tag="bc")
                        nc.tensor.matmul(bc, ones_sb, den, start=True, stop=True)
                        rbc = small_pool.tile([64, 512], FP32, name="rbc", tag="rbc")
                        nc.vector.reciprocal_approx_fast(rbc, bc)
                        nc.vector.tensor_mul(
                            onorm[pr][
                                hh * 64 : hh * 64 + 64, qt * 512 : (qt + 1) * 512
                            ],
                            raw,
                            rbc,
                        )

                    misc.append(norm_tail)
                if pr == PAIRS - 1:
                    for tb in range(qt * 4, qt * 4 + 4):
                        for nh in (0, 1):
                            misc.append(outproj_unit(tb, nh))

            return go

        for pr in range(PAIRS):
            if pr + 1 < PAIRS:
                misc.extend(emit_qk(pr + 1))
            for qt in range(QT):
                nkb = 4 * qt + 4
                outps = [
                    apsum.tile([128, 512], FP32, name=f"outp{hh}", tag=f"av{hh}")
                    for hh in (0, 1)
                ]
                # 1-k-block groups: off-diagonal (full 512 q cols), then the 4
                # diagonal sub-blocks (column-trimmed): (kb, ncols, qcol0, diag)
                subs = [(kb, 512, 0, False) for kb in range(4 * qt)]
                subs += [(4 * qt + j, 512 - 128 * j, 128 * j, True) for j in range(4)]

                for si, (kb, ncols, qcol0, diag) in enumerate(subs):
                    # scores for both head-halves packed into one psum tile:
                    # hh0 at cols [0, ncols), hh1 at cols [512, 512+ncols)
                    sps = spsum.tile([128, 1024], FP32, name="sps", tag="sc")
                    for hh in (0, 1):
                        nc.tensor.matmul(
                            sps[:, hh * 512 : hh * 512 + ncols],
                            kst[pr][hh * 64 : hh * 64 + 64, kb * 128 : (kb + 1) * 128],
                            qst[pr][
                                hh * 64 : hh * 64 + 64,
                                qt * 512 + qcol0 : qt * 512 + qcol0 + ncols,
                            ],
                            start=True,
                            stop=True,
                            tile_position=(hh * 64, 0),
                        )
                    pexp = p_pool.tile([128, 1024], BF16, name="pexp", tag="p")
                    if ncols == 512:
                        nc.scalar.activation(pexp, sps, Exp, scale=float(SCALE))
                    else:
                        for hh in (0, 1):
                            nc.scalar.activation(
                                pexp[:, hh * 512 : hh * 512 + ncols],
                                sps[:, hh * 512 : hh * 512 + ncols],
                                Exp,
                                scale=float(SCALE),
                            )
                    if diag:
                        # zero the strictly-upper triangle of the 128-wide
                        # diagonal window (post-exp 0/1 mask)
                        for hh in (0, 1):
                            nc.vector.tensor_mul(
                                pexp[:, hh * 512 : hh * 512 + 128],
                                pexp[:, hh * 512 : hh * 512 + 128],
                                mask_sb,
                            )
                    if pending is not None:
                        pending()

                    def attnv(
                        pexp=pexp,
                        kb=kb,
                        ncols=ncols,
                        qcol0=qcol0,
                        outps=outps,
                        pr=pr,
                        first=(kb == 0),
                        last=(kb == nkb - 1),
                    ):
                        vs = v_sb[kb].rearrange("p (h e) -> p h e", e=65)
                        for hh in (0, 1):
                            nc.tensor.matmul(
                                outps[hh][0:65, qcol0 : qcol0 + ncols],
                                vs[:, 2 * pr + hh, :],
                                pexp[:, hh * 512 : hh * 512 + ncols],
                                start=first,
                                stop=last,
                            )

                    pending = attnv
                    if misc:
                        misc.popleft()()

                make_finalize(pr, qt, outps, pending)()
                pending = None

        while misc:
            misc.popleft()()

    if not nc.is_finalized():
        nc.finalize()
    return nc


def shard_inputs(x, qkv_w, qkv_b, out_w):
    """Build the 8 per-core input maps (host-side bf16 casts)."""
    x = np.asarray(x, dtype=np.float32)
    qkv_w = np.asarray(qkv_w, dtype=np.float32)
    qkv_b = np.asarray(qkv_b, dtype=np.float32)
    out_w = np.asarray(out_w, dtype=np.float32)

    # 0/1 lower-triangular keep-mask for the post-exp diagonal-window zeroing
    mask = (
        (np.arange(128)[:, None] <= np.arange(128)[None, :])
        .astype(BF16_NP)
    )

    in_maps = []
    for core in range(N_CORES):
        b, hg = core // HG, core % HG
        col0 = hg * 512
        wq_np = np.ascontiguousarray(qkv_w[:, col0 : col0 + 512]).astype(BF16_NP)
        wk_np = np.ascontiguousarray(qkv_w[:, C + col0 : C + col0 + 512]).astype(
            BF16_NP
        )
        wv_np = np.ascontiguousarray(
            qkv_w[:, 2 * C + col0 : 2 * C + col0 + 512]
        ).astype(BF16_NP)
        bq_np = np.ascontiguousarray(qkv_b[col0 : col0 + 512].reshape(PAIRS, 128).T)
        bk_np = np.ascontiguousarray(
            qkv_b[C + col0 : C + col0 + 512].reshape(PAIRS, 128).T
        )
        bv_np = np.ascontiguousarray(
            qkv_b[2 * C + col0 : 2 * C + col0 + 512].reshape(1, 512)
        )
        wo_np = np.ascontiguousarray(out_w[col0 : col0 + 512, :]).astype(BF16_NP)
        xT_np = np.ascontiguousarray(x[b].T).astype(BF16_NP)
        in_maps.append(
            {
                "xT": xT_np,
                "wq": wq_np,
                "wk": wk_np,
                "wv": wv_np,
                "bq": bq_np,
                "bk": bk_np,
                "bv": bv_np,
                "wo": wo_np,
                "mask": mask,
            }
        )
    return in_maps


def kernel(x, qkv_w, qkv_b, out_w, out_b, _trace=False, _tmpdir=None):
    if "nc" not in _program_cache:
        _program_cache["nc"] = build_program()
    nc = _program_cache["nc"]

    in_maps = shard_inputs(x, qkv_w, qkv_b, out_w)
    res = run_bass_kernel_spmd(
        nc,
        in_maps,
        core_ids=list(range(N_CORES)),
        trace=_trace,
        tmpdir=_tmpdir,
    )
    _program_cache["last_results"] = res

    out_b = np.asarray(out_b, dtype=np.float32)
    y = np.empty((B, T, C), dtype=np.float32)
    for b in range(B):
        y[b] = (
            res.results[2 * b]["y"].astype(np.float32)
            + res.results[2 * b + 1]["y"].astype(np.float32)
            + out_b
        )
    return y


# revision 21
# speedup vs baseline: 3.7201x; 1.2432x over previous
"""Multi-head causal attention block (B=4, T=2048, C=1024, H=16) on 8 TRN2 cores.

Sharding: core c handles batch b = c // 2 and head-group hg = c % 2 (8 heads).
Each core computes q/k/v for its 8 heads from x[b], runs causal attention, and
produces a partial output-projection y_partial[b] = attnout @ out_w[rows_hg].
Host sums the two head-group partials per batch (fp32) and adds out_b.

All matmul operands are bf16 (fp32 matmul is 2-pass on the PE; bf16 is
single-pass => 2x tensor throughput), accumulation stays fp32 in PSUM.
Feature-major layout: x is fed as xT = x[b].T so the QKV projection needs no
on-device transposes; q/k come out d-major with head pairs stacked on
partitions 0-63 / 64-127 (row-packed K=64 score matmuls via tile_position);
v comes out token-major which is the lhsT layout for attn@V. A ones column
appended to each V tile yields the softmax denominator on row 64 of the
attn@V output.

Pipeline: attention runs as a stream of 1-k-block groups with a one-group
lookahead (scores of group g+1 issue before attn@V of group g) so the PE
never head-blocks waiting on the ScalarE exp. Q/K projection of pair p+1 and
the output projection are interleaved into the stream as PE filler. Softmax
normalization uses a K=1 outer-product matmul to broadcast den across
partitions, then a fast approximate reciprocal (no serial [1,512] recip, no
DRAM bounce). Input/weight DMAs are merged into few 3D-AP transfers because
each DMA_DIRECT2D costs ~700ns of Sync-engine issue time.
"""

import os
import sys
from collections import deque
from contextlib import ExitStack

import numpy as np
import ml_dtypes

for _p in ("/opt/trn_rl_repo", "/root/.axon_site/_ro/trn_rl_repo"):
    if os.path.isdir(_p) and _p not in sys.path:
        sys.path.insert(0, _p)

import concourse.bass as bass
import concourse.bacc as bacc
import concourse.mybir as mybir
import concourse.tile as tile
from concourse.bass_utils import run_bass_kernel_spmd

B, T, C, H = 4, 2048, 1024, 16
D = C // H  # 64
N_CORES = 8
HG = 2  # head groups per batch (cores per batch)
HPG = H // HG  # 8 heads per core
PAIRS = HPG // 2  # 4 head pairs per core
TB = T // 128  # 16 token blocks
QT = T // 512  # 4 q tiles
CT = C // 128  # 8 contraction tiles
FP32 = mybir.dt.float32
BF16 = mybir.dt.bfloat16
BF16_NP = ml_dtypes.bfloat16
SCALE = 1.0 / np.sqrt(np.float32(D))

_program_cache = {}


def build_program():
    nc = bacc.Bacc("TRN2", target_bir_lowering=False, debug=False, num_devices=N_CORES)

    xT = nc.declare_dram_parameter("xT", [C, T], BF16, isOutput=False)
    wq = nc.declare_dram_parameter("wq", [C, 512], BF16, isOutput=False)
    wk = nc.declare_dram_parameter("wk", [C, 512], BF16, isOutput=False)
    wv = nc.declare_dram_parameter("wv", [C, 512], BF16, isOutput=False)
    bq = nc.declare_dram_parameter("bq", [128, PAIRS], FP32, isOutput=False)
    bk = nc.declare_dram_parameter("bk", [128, PAIRS], FP32, isOutput=False)
    bv = nc.declare_dram_parameter("bv", [1, 512], FP32, isOutput=False)
    wo = nc.declare_dram_parameter("wo", [512, C], BF16, isOutput=False)
    maskp = nc.declare_dram_parameter("mask", [128, 128], BF16, isOutput=False)
    y = nc.declare_dram_parameter("y", [T, C], BF16, isOutput=True)

    Exp = mybir.ActivationFunctionType.Exp

    with tile.TileContext(nc) as tc, ExitStack() as ctx:
        persist = ctx.enter_context(tc.tile_pool(name="persist", bufs=1))

        mask_sb = persist.tile([128, 128], BF16, name="mask_sb", tag="mask_sb")
        nc.sync.dma_start(mask_sb, maskp[:, :])
        bq_sb = persist.tile([128, PAIRS], FP32, name="bq_sb", tag="bq_sb")
        nc.sync.dma_start(bq_sb, bq[:, :])
        bk_sb = persist.tile([128, PAIRS], FP32, name="bk_sb", tag="bk_sb")
        nc.sync.dma_start(bk_sb, bk[:, :])
        ones_sb = persist.tile([1, 64], BF16, name="ones_sb", tag="ones_sb")
        nc.vector.memset(ones_sb, 1.0)

        # V with interleaved ones columns: per token-block, [128, 8*65] where
        # group h holds V[:, h*64:(h+1)*64] | 1.
        v_sb = [
            persist.tile([128, HPG * 65], BF16, name=f"v_sb{i}", tag=f"v_sb{i}")
            for i in range(TB)
        ]
        qst = [
            persist.tile([128, T], BF16, name=f"qst{p}", tag=f"qst{p}")
            for p in range(PAIRS)
        ]
        kst = [
            persist.tile([128, T], BF16, name=f"kst{p}", tag=f"kst{p}")
            for p in range(PAIRS)
        ]
        onorm = [
            persist.tile([128, T], BF16, name=f"onorm{p}", tag=f"on{p}")
            for p in range(PAIRS)
        ]
        # merged weight/input tiles (middle index = contraction/pair block)
        wo_all = persist.tile([128, PAIRS, C], BF16, name="wo_all", tag="wo_all")
        xt_all = persist.tile([128, CT, T], BF16, name="xt_all", tag="xt_all")
        wv_all = persist.tile([128, CT, 512], BF16, name="wv_all", tag="wv_all")
        bv_bc = persist.tile([128, 512], FP32, name="bv_bc", tag="bv_bc")
        nc.sync.dma_start(bv_bc, bv[:, :].to_broadcast([128, 512]))

        # DMA order: wv first (V pass needs it), then xT column-major so
        # early token blocks land first, wo last.
        for i in range(CT):
            nc.sync.dma_start(wv_all[:, i, :], wv[i * 128 : (i + 1) * 128, :])
        for c in range(T // 512):
            for i in range(CT):
                nc.sync.dma_start(
                    xt_all[:, i, c * 512 : (c + 1) * 512],
                    xT[i * 128 : (i + 1) * 128, c * 512 : (c + 1) * 512],
                )
        for p in range(PAIRS):
            nc.sync.dma_start(wo_all[:, p, :], wo[p * 128 : (p + 1) * 128, :])

        wqk_pool = ctx.enter_context(tc.tile_pool(name="wqk", bufs=2))
        p_pool = ctx.enter_context(tc.tile_pool(name="pexp", bufs=6))
        small_pool = ctx.enter_context(tc.tile_pool(name="small", bufs=4))

        # ---------------- V pass (scoped PSUM pool) ----------------
        vctx = ExitStack()
        vpsum = vctx.enter_context(tc.tile_pool(name="vpsum", bufs=3, space="PSUM"))
        for tb in range(TB):
            pv = vpsum.tile([128, 512], FP32, name="pv", tag="pv")
            for ci in range(CT):
                nc.tensor.matmul(
                    pv,
                    xt_all[:, ci, tb * 128 : (tb + 1) * 128],
                    wv_all[:, ci, :],
                    start=(ci == 0),
                    stop=(ci == CT - 1),
                )
            vt = v_sb[tb].rearrange("p (h e) -> p h e", e=65)
            nc.vector.tensor_add(
                vt[:, :, 0:64],
                pv.rearrange("p (h e) -> p h e", e=64),
                bv_bc.rearrange("p (h e) -> p h e", e=64),
            )
            nc.vector.memset(vt[:, :, 64:65], 1.0)
        vctx.close()

        # ---- Q/K projection emitted as closures (interleavable) ----
        def emit_qk(pr, pool):
            closures = []
            for wdram, bias_sb, dst in ((wq, bq_sb, qst[pr]), (wk, bk_sb, kst[pr])):
                box = {}

                def load_w(box=box, wdram=wdram, pr=pr):
                    w_all = wqk_pool.tile(
                        [128, CT, 128], BF16, name="w_all", tag="w_all"
                    )
                    for ci in range(CT):
                        nc.sync.dma_start(
                            w_all[:, ci, :],
                            wdram[ci * 128 : (ci + 1) * 128, pr * 128 : (pr + 1) * 128],
                        )
                    box["w"] = w_all

                closures.append(load_w)

                def chunk(qt, box=box, bias_sb=bias_sb, dst=dst, pr=pr, pool=pool):
                    def go():
                        pq = pool.tile([128, 512], FP32, name="pq", tag="pq")
                        w_all = box["w"]
                        for ci in range(CT):
                            nc.tensor.matmul(
                                pq,
                                w_all[:, ci, :],
                                xt_all[:, ci, qt * 512 : (qt + 1) * 512],
                                start=(ci == 0),
                                stop=(ci == CT - 1),
                            )
                        nc.vector.tensor_scalar_add(
                            dst[:, qt * 512 : (qt + 1) * 512],
                            pq,
                            bias_sb[:, pr : pr + 1],
                        )

                    return go

                closures.extend(chunk(qt) for qt in range(QT))
            return closures

        # Q/K for pair 0 runs up front with its own double-buffered psum pool
        # (closed before the attention pools open).
        qk0ctx = ExitStack()
        qk0psum = qk0ctx.enter_context(
            tc.tile_pool(name="qk0psum", bufs=2, space="PSUM")
        )
        for cl in emit_qk(0, qk0psum):
            cl()
        qk0ctx.close()

        # ---------------- attention-phase PSUM pools ----------------
        # budget: sps 2x[128,1024] = 4 banks, outps 2x[128,512] = 2, pq 1,
        # bc 1 -> 8 banks exactly.
        pqp = ctx.enter_context(tc.tile_pool(name="pqp", bufs=1, space="PSUM"))
        spsum = ctx.enter_context(tc.tile_pool(name="spsum", bufs=2, space="PSUM"))
        apsum = ctx.enter_context(tc.tile_pool(name="apsum", bufs=1, space="PSUM"))
        bcp = ctx.enter_context(tc.tile_pool(name="bcp", bufs=1, space="PSUM"))

        def outproj_unit(tb, nh):
            def go():
                yp = pqp.tile([128, 512], FP32, name="yp", tag="pq")
                for p2 in range(PAIRS):
                    nc.tensor.matmul(
                        yp,
                        onorm[p2][:, tb * 128 : (tb + 1) * 128],
                        wo_all[:, p2, nh * 512 : (nh + 1) * 512],
                        start=(p2 == 0),
                        stop=(p2 == PAIRS - 1),
                    )
                ys = small_pool.tile([128, 512], BF16, name="ys", tag="ys")
                nc.vector.tensor_copy(ys, yp)
                nc.sync.dma_start(
                    y[tb * 128 : (tb + 1) * 128, nh * 512 : (nh + 1) * 512], ys
                )

            return go

        misc = deque()
        pending = None

        def make_finalize(pr, qt, outps, last_attnv):
            """Flush the tile's last attn@V, then free the outps banks with
            den/raw copies (DVE only), deferring the broadcast-matmul +
            approx-reciprocal + mul to the misc stream."""

            def go():
                last_attnv()
                for hh in (0, 1):
                    den = small_pool.tile([1, 512], BF16, name="den", tag="den")
                    nc.vector.tensor_copy(den, outps[hh][64:65, :])
                    raw = small_pool.tile([64, 512], BF16, name="raw", tag="raw")
                    nc.vector.tensor_copy(raw, outps[hh][0:64, :])

                    def norm_tail(den=den, raw=raw, pr=pr, qt=qt, hh=hh):
                        bc = bcp.tile([64, 512], FP32, name="bc", tag="bc")
                        nc.tensor.matmul(bc, ones_sb, den, start=True, stop=True)
                        rbc = small_pool.tile([64, 512], FP32, name="rbc", tag="rbc")
                        nc.vector.reciprocal_approx_fast(rbc, bc)
                        nc.vector.tensor_mul(
                            onorm[pr][
                                hh * 64 : hh * 64 + 64, qt * 512 : (qt + 1) * 512
                            ],
                            raw,
                            rbc,
                        )

                    misc.append(norm_tail)
                if pr == PAIRS - 1:
                    for tb in range(qt * 4, qt * 4 + 4):
                        for nh in (0, 1):
                            misc.append(outproj_unit(tb, nh))

            return go

        for pr in range(PAIRS):
            if pr + 1 < PAIRS:
                misc.extend(emit_qk(pr + 1, pqp))
            for qt in range(QT):
                nkb = 4 * qt + 4
                outps = [
                    apsum.tile([128, 512], FP32, name=f"outp{hh}", tag=f"av{hh}")
                    for hh in (0, 1)
                ]
                # 1-k-block groups: off-diagonal (full 512 q cols), then the 4
                # diagonal sub-blocks (column-trimmed). col1 = hh1's column
                # offset in the score/pexp tile — always 512 (bank 1): the two
                # head-halves run CONCURRENTLY on different PE row groups, so
                # they must drain into different PSUM banks.
                subs = [(kb, 512, 0, 512, False) for kb in range(4 * qt)]
                subs += [
                    (4 * qt + j, 512 - 128 * j, 128 * j, 512, True) for j in range(4)
                ]

                for si, (kb, ncols, qcol0, col1, diag) in enumerate(subs):
                    sps = spsum.tile([128, 1024], FP32, name="sps", tag="sc")
                    for hh in (0, 1):
                        c0 = hh * col1
                        nc.tensor.matmul(
                            sps[:, c0 : c0 + ncols],
                            kst[pr][hh * 64 : hh * 64 + 64, kb * 128 : (kb + 1) * 128],
                            qst[pr][
                                hh * 64 : hh * 64 + 64,
                                qt * 512 + qcol0 : qt * 512 + qcol0 + ncols,
                            ],
                            start=True,
                            stop=True,
                            tile_position=(hh * 64, 0),
                        )
                    pexp = p_pool.tile([128, 1024], BF16, name="pexp", tag="p")
                    if col1 == ncols or ncols == 512:
                        nc.scalar.activation(
                            pexp[:, 0 : col1 + ncols],
                            sps[:, 0 : col1 + ncols],
                            Exp,
                            scale=float(SCALE),
                        )
                    else:
                        for hh in (0, 1):
                            c0 = hh * col1
                            nc.scalar.activation(
                                pexp[:, c0 : c0 + ncols],
                                sps[:, c0 : c0 + ncols],
                                Exp,
                                scale=float(SCALE),
                            )
                    if diag:
                        # zero the strictly-upper triangle of the 128-wide
                        # diagonal window (post-exp 0/1 mask)
                        for hh in (0, 1):
                            c0 = hh * col1
                            nc.vector.tensor_mul(
                                pexp[:, c0 : c0 + 128],
                                pexp[:, c0 : c0 + 128],
                                mask_sb,
                            )
                    if pending is not None:
                        pending()

                    def attnv(
                        pexp=pexp,
                        kb=kb,
                        ncols=ncols,
                        qcol0=qcol0,
                        col1=col1,
                        outps=outps,
                        pr=pr,
                        first=(kb == 0),
                        last=(kb == nkb - 1),
                    ):
                        vs = v_sb[kb].rearrange("p (h e) -> p h e", e=65)
                        for hh in (0, 1):
                            c0 = hh * col1
                            nc.tensor.matmul(
                                outps[hh][0:65, qcol0 : qcol0 + ncols],
                                vs[:, 2 * pr + hh, :],
                                pexp[:, c0 : c0 + ncols],
                                start=first,
                                stop=last,
                            )

                    pending = attnv
                    if misc:
                        misc.popleft()()

                # one filler unit while the last group's exp completes, then
                # flush + evacuate
                if misc:
                    misc.popleft()()
                make_finalize(pr, qt, outps, pending)()
                pending = None

        while misc:
            misc.popleft()()

    if not nc.is_finalized():
        nc.finalize()
    return nc


def shard_inputs(x, qkv_w, qkv_b, out_w):
    """Build the 8 per-core input maps (host-side bf16 casts)."""
    x = np.asarray(x, dtype=np.float32)
    qkv_w = np.asarray(qkv_w, dtype=np.float32)
    qkv_b = np.asarray(qkv_b, dtype=np.float32)
    out_w = np.asarray(out_w, dtype=np.float32)

    # 0/1 lower-triangular keep-mask for the post-exp diagonal-window zeroing
    mask = (np.arange(128)[:, None] <= np.arange(128)[None, :]).astype(BF16_NP)

    in_maps = []
    for core in range(N_CORES):
        b, hg = core // HG, core % HG
        col0 = hg * 512
        wq_np = np.ascontiguousarray(qkv_w[:, col0 : col0 + 512]).astype(BF16_NP)
        wk_np = np.ascontiguousarray(qkv_w[:, C + col0 : C + col0 + 512]).astype(
            BF16_NP
        )
        wv_np = np.ascontiguousarray(
            qkv_w[:, 2 * C + col0 : 2 * C + col0 + 512]
        ).astype(BF16_NP)
        bq_np = np.ascontiguousarray(qkv_b[col0 : col0 + 512].reshape(PAIRS, 128).T)
        bk_np = np.ascontiguousarray(
            qkv_b[C + col0 : C + col0 + 512].reshape(PAIRS, 128).T
        )
        bv_np = np.ascontiguousarray(
            qkv_b[2 * C + col0 : 2 * C + col0 + 512].reshape(1, 512)
        )
        wo_np = np.ascontiguousarray(out_w[col0 : col0 + 512, :]).astype(BF16_NP)
        xT_np = np.ascontiguousarray(x[b].T).astype(BF16_NP)
        in_maps.append(
            {
                "xT": xT_np,
                "wq": wq_np,
                "wk": wk_np,
                "wv": wv_np,
                "bq": bq_np,
                "bk": bk_np,
                "bv": bv_np,
                "wo": wo_np,
                "mask": mask,
            }
        )
    return in_maps


def kernel(x, qkv_w, qkv_b, out_w, out_b, _trace=False, _tmpdir=None):
    if "nc" not in _program_cache:
        _program_cache["nc"] = build_program()
    nc = _program_cache["nc"]

    in_maps = shard_inputs(x, qkv_w, qkv_b, out_w)
    res = run_bass_kernel_spmd(
        nc,
        in_maps,
        core_ids=list(range(N_CORES)),
        trace=_trace,
        tmpdir=_tmpdir,
    )
    _program_cache["last_results"] = res

    out_b = np.asarray(out_b, dtype=np.float32)
    y = np.empty((B, T, C), dtype=np.float32)
    for b in range(B):
        y[b] = (
            res.results[2 * b]["y"].astype(np.float32)
            + res.results[2 * b + 1]["y"].astype(np.float32)
            + out_b
        )
    return y


# revision 23
# speedup vs baseline: 3.8013x; 1.0218x over previous
"""Multi-head causal attention block (B=4, T=2048, C=1024, H=16) on 8 TRN2 cores.

Sharding: core c handles batch b = c // 2 and head-group hg = c % 2 (8 heads).
Each core computes q/k/v for its 8 heads from x[b], runs causal attention, and
produces a partial output-projection y_partial[b] = attnout @ out_w[rows_hg].
Host sums the two head-group partials per batch (fp32) and adds out_b.

All matmul operands are bf16 (fp32 matmul is 2-pass on the PE; bf16 is
single-pass => 2x tensor throughput), accumulation stays fp32 in PSUM.
Feature-major layout: x is fed as xT = x[b].T so the QKV projection needs no
on-device transposes; q/k come out d-major with head pairs stacked on
partitions 0-63 / 64-127 (row-packed K=64 score matmuls via tile_position);
v comes out token-major which is the lhsT layout for attn@V. A ones column
appended to each V tile yields the softmax denominator on row 64 of the
attn@V output.

Pipeline: attention runs as a stream of 1-k-block groups with a one-group
lookahead (scores of group g+1 issue before attn@V of group g) so the PE
never head-blocks waiting on the ScalarE exp. Q/K projection of pair p+1 and
the output projection are interleaved into the stream as PE filler. Softmax
normalization uses a K=1 outer-product matmul to broadcast den across
partitions, then a fast approximate reciprocal (no serial [1,512] recip, no
DRAM bounce). Input/weight DMAs are merged into few 3D-AP transfers because
each DMA_DIRECT2D costs ~700ns of Sync-engine issue time.
"""

import os
import sys
from collections import deque
from contextlib import ExitStack

import numpy as np
import ml_dtypes

for _p in ("/opt/trn_rl_repo", "/root/.axon_site/_ro/trn_rl_repo"):
    if os.path.isdir(_p) and _p not in sys.path:
        sys.path.insert(0, _p)

import concourse.bass as bass
import concourse.bacc as bacc
import concourse.mybir as mybir
import concourse.tile as tile
from concourse.bass_utils import run_bass_kernel_spmd

B, T, C, H = 4, 2048, 1024, 16
D = C // H  # 64
N_CORES = 8
HG = 2  # head groups per batch (cores per batch)
HPG = H // HG  # 8 heads per core
PAIRS = HPG // 2  # 4 head pairs per core
TB = T // 128  # 16 token blocks
QT = T // 512  # 4 q tiles
CT = C // 128  # 8 contraction tiles
FP32 = mybir.dt.float32
BF16 = mybir.dt.bfloat16
BF16_NP = ml_dtypes.bfloat16
SCALE = 1.0 / np.sqrt(np.float32(D))

_program_cache = {}


def build_program():
    nc = bacc.Bacc("TRN2", target_bir_lowering=False, debug=False, num_devices=N_CORES)

    xT = nc.declare_dram_parameter("xT", [C, T], BF16, isOutput=False)
    wq = nc.declare_dram_parameter("wq", [C, 512], BF16, isOutput=False)
    wk = nc.declare_dram_parameter("wk", [C, 512], BF16, isOutput=False)
    wv = nc.declare_dram_parameter("wv", [C, 512], BF16, isOutput=False)
    bq = nc.declare_dram_parameter("bq", [128, PAIRS], FP32, isOutput=False)
    bk = nc.declare_dram_parameter("bk", [128, PAIRS], FP32, isOutput=False)
    bv = nc.declare_dram_parameter("bv", [1, 512], FP32, isOutput=False)
    wo = nc.declare_dram_parameter("wo", [512, C], BF16, isOutput=False)
    maskp = nc.declare_dram_parameter("mask", [128, 128], BF16, isOutput=False)
    y = nc.declare_dram_parameter("y", [T, C], BF16, isOutput=True)

    Exp = mybir.ActivationFunctionType.Exp

    with tile.TileContext(nc) as tc, ExitStack() as ctx:
        persist = ctx.enter_context(tc.tile_pool(name="persist", bufs=1))

        mask_sb = persist.tile([128, 128], BF16, name="mask_sb", tag="mask_sb")
        nc.sync.dma_start(mask_sb, maskp[:, :])
        bq_sb = persist.tile([128, PAIRS], FP32, name="bq_sb", tag="bq_sb")
        nc.sync.dma_start(bq_sb, bq[:, :])
        bk_sb = persist.tile([128, PAIRS], FP32, name="bk_sb", tag="bk_sb")
        nc.sync.dma_start(bk_sb, bk[:, :])
        ones_sb = persist.tile([1, 64], BF16, name="ones_sb", tag="ones_sb")
        nc.vector.memset(ones_sb, 1.0)

        # V with interleaved ones columns: per token-block, [128, 8*65] where
        # group h holds V[:, h*64:(h+1)*64] | 1.
        v_sb = [
            persist.tile([128, HPG * 65], BF16, name=f"v_sb{i}", tag=f"v_sb{i}")
            for i in range(TB)
        ]
        qst = [
            persist.tile([128, T], BF16, name=f"qst{p}", tag=f"qst{p}")
            for p in range(PAIRS)
        ]
        kst = [
            persist.tile([128, T], BF16, name=f"kst{p}", tag=f"kst{p}")
            for p in range(PAIRS)
        ]
        onorm = [
            persist.tile([128, T], BF16, name=f"onorm{p}", tag=f"on{p}")
            for p in range(PAIRS)
        ]
        # merged weight/input tiles (middle index = contraction/pair block)
        wo_all = persist.tile([128, PAIRS, C], BF16, name="wo_all", tag="wo_all")
        xt_all = persist.tile([128, CT, T], BF16, name="xt_all", tag="xt_all")
        wv_all = persist.tile([128, CT, 512], BF16, name="wv_all", tag="wv_all")
        bv_bc = persist.tile([128, 512], FP32, name="bv_bc", tag="bv_bc")
        nc.sync.dma_start(bv_bc, bv[:, :].to_broadcast([128, 512]))

        # Merged DMAs (each DMA_DIRECT2D costs ~700ns of Sync-engine issue
        # time): wv first (V pass needs it), then xT column-major so early
        # token blocks land first, wo last.
        for g in (0, 1):
            nc.sync.dma_start(
                wv_all[:, g * 4 : (g + 1) * 4, :],
                wv[g * 512 : (g + 1) * 512, :].rearrange("(a p) f -> p a f", p=128),
            )
        for c in range(T // 512):
            for g in (0, 1):
                nc.sync.dma_start(
                    xt_all[:, g * 4 : (g + 1) * 4, c * 512 : (c + 1) * 512],
                    xT[
                        g * 512 : (g + 1) * 512, c * 512 : (c + 1) * 512
                    ].rearrange("(a p) f -> p a f", p=128),
                )
        for g in (0, 1):
            nc.sync.dma_start(
                wo_all[:, g * 2 : (g + 1) * 2, :],
                wo[g * 256 : (g + 1) * 256, :].rearrange("(a p) f -> p a f", p=128),
            )

        wqk_pool = ctx.enter_context(tc.tile_pool(name="wqk", bufs=2))
        p_pool = ctx.enter_context(tc.tile_pool(name="pexp", bufs=6))
        small_pool = ctx.enter_context(tc.tile_pool(name="small", bufs=4))

        # ---------------- V pass (scoped PSUM pool) ----------------
        vctx = ExitStack()
        vpsum = vctx.enter_context(tc.tile_pool(name="vpsum", bufs=3, space="PSUM"))
        for tb in range(TB):
            pv = vpsum.tile([128, 512], FP32, name="pv", tag="pv")
            for ci in range(CT):
                nc.tensor.matmul(
                    pv,
                    xt_all[:, ci, tb * 128 : (tb + 1) * 128],
                    wv_all[:, ci, :],
                    start=(ci == 0),
                    stop=(ci == CT - 1),
                )
            vt = v_sb[tb].rearrange("p (h e) -> p h e", e=65)
            nc.vector.tensor_add(
                vt[:, :, 0:64],
                pv.rearrange("p (h e) -> p h e", e=64),
                bv_bc.rearrange("p (h e) -> p h e", e=64),
            )
            nc.vector.memset(vt[:, :, 64:65], 1.0)
        vctx.close()

        # ---- Q/K projection emitted as closures (interleavable) ----
        def emit_qk(pr, pool):
            closures = []
            for wdram, bias_sb, dst in ((wq, bq_sb, qst[pr]), (wk, bk_sb, kst[pr])):
                box = {}

                def load_w(box=box, wdram=wdram, pr=pr):
                    w_all = wqk_pool.tile(
                        [128, CT, 128], BF16, name="w_all", tag="w_all"
                    )
                    for ci in range(CT):
                        nc.sync.dma_start(
                            w_all[:, ci, :],
                            wdram[ci * 128 : (ci + 1) * 128, pr * 128 : (pr + 1) * 128],
                        )
                    box["w"] = w_all

                closures.append(load_w)

                def chunk(qt, box=box, bias_sb=bias_sb, dst=dst, pr=pr, pool=pool):
                    def go():
                        pq = pool.tile([128, 512], FP32, name="pq", tag="pq")
                        w_all = box["w"]
                        for ci in range(CT):
                            nc.tensor.matmul(
                                pq,
                                w_all[:, ci, :],
                                xt_all[:, ci, qt * 512 : (qt + 1) * 512],
                                start=(ci == 0),
                                stop=(ci == CT - 1),
                            )
                        nc.vector.tensor_scalar_add(
                            dst[:, qt * 512 : (qt + 1) * 512],
                            pq,
                            bias_sb[:, pr : pr + 1],
                        )

                    return go

                closures.extend(chunk(qt) for qt in range(QT))
            return closures

        # Q/K for pair 0 runs up front with its own double-buffered psum pool
        # (closed before the attention pools open).
        qk0ctx = ExitStack()
        qk0psum = qk0ctx.enter_context(
            tc.tile_pool(name="qk0psum", bufs=2, space="PSUM")
        )
        for cl in emit_qk(0, qk0psum):
            cl()
        qk0ctx.close()

        # ---------------- attention-phase PSUM pools ----------------
        # budget: sps 2x[128,1024] = 4 banks, outps 2x[128,512] = 2, pq 1,
        # bc 1 -> 8 banks exactly.
        pqp = ctx.enter_context(tc.tile_pool(name="pqp", bufs=1, space="PSUM"))
        spsum = ctx.enter_context(tc.tile_pool(name="spsum", bufs=2, space="PSUM"))
        apsum = ctx.enter_context(tc.tile_pool(name="apsum", bufs=1, space="PSUM"))
        bcp = ctx.enter_context(tc.tile_pool(name="bcp", bufs=1, space="PSUM"))

        def outproj_unit(tb, nh):
            def go():
                # alternate psum slots (pq / bc share 2KB-per-partition slot
                # sizes) so back-to-back units at the tail double-buffer
                if (2 * tb + nh) % 2:
                    yp = bcp.tile([128, 512], FP32, name="yp", tag="bc")
                else:
                    yp = pqp.tile([128, 512], FP32, name="yp", tag="pq")
                for p2 in range(PAIRS):
                    nc.tensor.matmul(
                        yp,
                        onorm[p2][:, tb * 128 : (tb + 1) * 128],
                        wo_all[:, p2, nh * 512 : (nh + 1) * 512],
                        start=(p2 == 0),
                        stop=(p2 == PAIRS - 1),
                    )
                ys = small_pool.tile([128, 512], BF16, name="ys", tag="ys")
                nc.vector.tensor_copy(ys, yp)
                nc.sync.dma_start(
                    y[tb * 128 : (tb + 1) * 128, nh * 512 : (nh + 1) * 512], ys
                )

            return go

        misc = deque()
        pending = None

        def make_finalize(pr, qt, outps, last_attnv):
            """Flush the tile's last attn@V, then free the outps banks with
            den/raw copies (DVE only), deferring the broadcast-matmul +
            approx-reciprocal + mul to the misc stream."""

            def go():
                last_attnv()
                for hh in (0, 1):
                    den = small_pool.tile([1, 512], BF16, name="den", tag="den")
                    nc.vector.tensor_copy(den, outps[hh][64:65, :])
                    raw = small_pool.tile([64, 512], BF16, name="raw", tag="raw")
                    nc.vector.tensor_copy(raw, outps[hh][0:64, :])

                    def norm_tail(den=den, raw=raw, pr=pr, qt=qt, hh=hh):
                        bc = bcp.tile([64, 512], FP32, name="bc", tag="bc")
                        nc.tensor.matmul(bc, ones_sb, den, start=True, stop=True)
                        rbc = small_pool.tile([64, 512], FP32, name="rbc", tag="rbc")
                        nc.vector.reciprocal_approx_fast(rbc, bc)
                        nc.vector.tensor_mul(
                            onorm[pr][
                                hh * 64 : hh * 64 + 64, qt * 512 : (qt + 1) * 512
                            ],
                            raw,
                            rbc,
                        )

                    misc.append(norm_tail)
                if pr == PAIRS - 1:
                    for tb in range(qt * 4, qt * 4 + 4):
                        for nh in (0, 1):
                            misc.append(outproj_unit(tb, nh))

            return go

        for pr in range(PAIRS):
            if pr + 1 < PAIRS:
                misc.extend(emit_qk(pr + 1, pqp))
            for qt in range(QT):
                nkb = 4 * qt + 4
                outps = [
                    apsum.tile([128, 512], FP32, name=f"outp{hh}", tag=f"av{hh}")
                    for hh in (0, 1)
                ]
                # 1-k-block groups: off-diagonal (full 512 q cols), then the 4
                # diagonal sub-blocks (column-trimmed). col1 = hh1's column
                # offset in the score/pexp tile — always 512 (bank 1): the two
                # head-halves run CONCURRENTLY on different PE row groups, so
                # they must drain into different PSUM banks.
                subs = [(kb, 512, 0, 512, False) for kb in range(4 * qt)]
                subs += [
                    (4 * qt + j, 512 - 128 * j, 128 * j, 512, True) for j in range(4)
                ]

                for si, (kb, ncols, qcol0, col1, diag) in enumerate(subs):
                    sps = spsum.tile([128, 1024], FP32, name="sps", tag="sc")
                    for hh in (0, 1):
                        c0 = hh * col1
                        nc.tensor.matmul(
                            sps[:, c0 : c0 + ncols],
                            kst[pr][hh * 64 : hh * 64 + 64, kb * 128 : (kb + 1) * 128],
                            qst[pr][
                                hh * 64 : hh * 64 + 64,
                                qt * 512 + qcol0 : qt * 512 + qcol0 + ncols,
                            ],
                            start=True,
                            stop=True,
                            tile_position=(hh * 64, 0),
                        )
                    pexp = p_pool.tile([128, 1024], BF16, name="pexp", tag="p")
                    if col1 == ncols or ncols == 512:
                        nc.scalar.activation(
                            pexp[:, 0 : col1 + ncols],
                            sps[:, 0 : col1 + ncols],
                            Exp,
                            scale=float(SCALE),
                        )
                    else:
                        for hh in (0, 1):
                            c0 = hh * col1
                            nc.scalar.activation(
                                pexp[:, c0 : c0 + ncols],
                                sps[:, c0 : c0 + ncols],
                                Exp,
                                scale=float(SCALE),
                            )
                    if diag:
                        # zero the strictly-upper triangle of the 128-wide
                        # diagonal window (post-exp 0/1 mask)
                        for hh in (0, 1):
                            c0 = hh * col1
                            nc.vector.tensor_mul(
                                pexp[:, c0 : c0 + 128],
                                pexp[:, c0 : c0 + 128],
                                mask_sb,
                            )
                    if pending is not None:
                        pending()

                    def attnv(
                        pexp=pexp,
                        kb=kb,
                        ncols=ncols,
                        qcol0=qcol0,
                        col1=col1,
                        outps=outps,
                        pr=pr,
                        first=(kb == 0),
                        last=(kb == nkb - 1),
                    ):
                        vs = v_sb[kb].rearrange("p (h e) -> p h e", e=65)
                        for hh in (0, 1):
                            c0 = hh * col1
                            nc.tensor.matmul(
                                outps[hh][0:65, qcol0 : qcol0 + ncols],
                                vs[:, 2 * pr + hh, :],
                                pexp[:, c0 : c0 + ncols],
                                start=first,
                                stop=last,
                            )

                    pending = attnv
                    if misc:
                        misc.popleft()()

                # one filler unit while the last group's exp completes, then
                # flush + evacuate
                if misc:
                    misc.popleft()()
                make_finalize(pr, qt, outps, pending)()
                pending = None

        while misc:
            misc.popleft()()

    if not nc.is_finalized():
        nc.finalize()
    return nc


def shard_inputs(x, qkv_w, qkv_b, out_w):
    """Build the 8 per-core input maps (host-side bf16 casts)."""
    x = np.asarray(x, dtype=np.float32)
    qkv_w = np.asarray(qkv_w, dtype=np.float32)
    qkv_b = np.asarray(qkv_b, dtype=np.float32)
    out_w = np.asarray(out_w, dtype=np.float32)

    # 0/1 lower-triangular keep-mask for the post-exp diagonal-window zeroing
    mask = (np.arange(128)[:, None] <= np.arange(128)[None, :]).astype(BF16_NP)

    in_maps = []
    for core in range(N_CORES):
        b, hg = core // HG, core % HG
        col0 = hg * 512
        wq_np = np.ascontiguousarray(qkv_w[:, col0 : col0 + 512]).astype(BF16_NP)
        wk_np = np.ascontiguousarray(qkv_w[:, C + col0 : C + col0 + 512]).astype(
            BF16_NP
        )
        wv_np = np.ascontiguousarray(
            qkv_w[:, 2 * C + col0 : 2 * C + col0 + 512]
        ).astype(BF16_NP)
        bq_np = np.ascontiguousarray(qkv_b[col0 : col0 + 512].reshape(PAIRS, 128).T)
        bk_np = np.ascontiguousarray(
            qkv_b[C + col0 : C + col0 + 512].reshape(PAIRS, 128).T
        )
        bv_np = np.ascontiguousarray(
            qkv_b[2 * C + col0 : 2 * C + col0 + 512].reshape(1, 512)
        )
        wo_np = np.ascontiguousarray(out_w[col0 : col0 + 512, :]).astype(BF16_NP)
        xT_np = np.ascontiguousarray(x[b].T).astype(BF16_NP)
        in_maps.append(
            {
                "xT": xT_np,
                "wq": wq_np,
                "wk": wk_np,
                "wv": wv_np,
                "bq": bq_np,
                "bk": bk_np,
                "bv": bv_np,
                "wo": wo_np,
                "mask": mask,
            }
        )
    return in_maps


def kernel(x, qkv_w, qkv_b, out_w, out_b, _trace=False, _tmpdir=None):
    if "nc" not in _program_cache:
        _program_cache["nc"] = build_program()
    nc = _program_cache["nc"]

    in_maps = shard_inputs(x, qkv_w, qkv_b, out_w)
    res = run_bass_kernel_spmd(
        nc,
        in_maps,
        core_ids=list(range(N_CORES)),
        trace=_trace,
        tmpdir=_tmpdir,
    )
    _program_cache["last_results"] = res

    out_b = np.asarray(out_b, dtype=np.float32)
    y = np.empty((B, T, C), dtype=np.float32)
    for b in range(B):
        y[b] = (
            res.results[2 * b]["y"].astype(np.float32)
            + res.results[2 * b + 1]["y"].astype(np.float32)
            + out_b
        )
    return y
